# revision 14
# baseline (speedup 1.0000x reference)
"""Chamfer distance (adv->ori direction) Trainium2 Bass kernel.

Problem: adv_pc [8, 4096, 3], ori_pc [8, 4096, 3], weights [8] ->
scalar f32 loss = mean_b( w_b * mean_k( min_j ||adv_bk - ori_bj||^2 ) ).

Sharding: data parallel over the batch dim — core b handles batch b.

Per-core algorithm (K = 4096 points, d = 3):
  m[k, j]   = b2_j - 2 a_k . b_j          (augmented matmul, contract dim 4)
  out_core  = sum_k ( a2_k + min_j m[k,j] )       (= 4096 * loss1_b)
The a2_k term is added per-point BEFORE the sum over k (the min is ~ -3.0
and a2 ~ +3.0; their sum is ~0.002, so summing them separately would lose
precision to cancellation).

Matmul form: lhsT = ahatT [4, 128]   rows (ax, ay, az, 1)   per k-tile
             rhs  = bhat  [4, 4096]  rows (-2bx, -2by, -2bz, b2)
             psum[t] = lhsT.T @ rhs  -> [128, j]   then DVE min-reduce over j.

The [coord, point] layouts are built on-chip with PE transposes of
[128 points, 4] tiles (identity-matmul), scaled during the PSUM->SBUF
copy on the Scalar engine (per-partition scale (-2,-2,-2,1) for ori).
"""

import numpy as np

B = 8
K = 4096
KT = K // 128  # 32 k-tiles of 128 adv points
NCORES = 8

_NC_CACHE = {}


def _build_nc():
    import concourse.bacc as bacc
    import concourse.mybir as mybir
    import concourse.tile as tile
    from concourse import masks

    f32 = mybir.dt.float32
    Alu = mybir.AluOpType
    Act = mybir.ActivationFunctionType
    Ax = mybir.AxisListType

    nc = bacc.Bacc("TRN2", target_bir_lowering=False, debug=False,
                   num_devices=NCORES)

    adv = nc.dram_tensor("adv", [K, 3], f32, kind="ExternalInput").ap()
    ori = nc.dram_tensor("ori", [K, 3], f32, kind="ExternalInput").ap()
    out = nc.dram_tensor("out", [1, 1], f32, kind="ExternalOutput").ap()

    with tile.TileContext(nc) as tc:
        with tc.tile_pool(name="consts", bufs=1) as consts, \
             tc.tile_pool(name="sb", bufs=1) as sb:
            ident = consts.tile([128, 128], f32)
            masks.make_identity(nc, ident[:])

            # Coord-major staging tiles [128, 128]: col 32q+t = coord q of
            # k-tile t (q=3 col block: 1.0 for adv, b2 for ori). Point
            # (128t + p) lives at row p, cols {t, 32+t, 64+t, 96+t}.
            Av = sb.tile([128, 4 * KT], f32)
            Ov = sb.tile([128, 4 * KT], f32)
            nc.gpsimd.memset(Av[:, 3 * KT:], 1.0)

            adv_v = adv.rearrange("(t p) d -> p d t", p=128)
            ori_v = ori.rearrange("(t p) d -> p d t", p=128)
            Av_v = Av[:, 0:3 * KT].rearrange("p (q t) -> p q t", q=3)
            Ov_v = Ov[:, 0:3 * KT].rearrange("p (q t) -> p q t", q=3)
            nc.sync.dma_start(out=Av_v[:], in_=adv_v[:])
            nc.sync.dma_start(out=Ov_v[:], in_=ori_v[:])

            # a2 per adv point -> a2arr [128, 32]; b2 per ori point -> col
            # 96+t of Ov (reduce over the stride-32 coord axis).
            Asq = sb.tile([128, 3 * KT], f32)
            Osq = sb.tile([128, 3 * KT], f32)
            a2arr = sb.tile([128, KT], f32)
            nc.vector.tensor_tensor(Asq[:], Av[:, 0:3 * KT], Av[:, 0:3 * KT],
                                    op=Alu.mult)
            nc.vector.tensor_tensor(Osq[:], Ov[:, 0:3 * KT], Ov[:, 0:3 * KT],
                                    op=Alu.mult)
            Asq_v = Asq[:].rearrange("p (q t) -> p t q", q=3)
            Osq_v = Osq[:].rearrange("p (q t) -> p t q", q=3)
            nc.vector.tensor_reduce(a2arr[:], Asq_v, axis=Ax.X, op=Alu.add)
            nc.vector.tensor_reduce(Ov[:, 3 * KT:], Osq_v, axis=Ax.X,
                                    op=Alu.add)
            # scale ori coords in place by -2 (b2 columns stay unscaled)
            nc.vector.tensor_scalar_mul(Ov[:, 0:3 * KT], Ov[:, 0:3 * KT],
                                        -2.0)

            # One PE transpose per tensor -> PSUM [128, 128] whose row 32q+t
            # holds coord q of k-tile t; engine-copy to SBUF, then DMAs
            # gather rows into the [4, K] operand layout and replicate it to
            # PE row groups 0/32/64/96 so four matmuls can run concurrently
            # via tile_position.
            ahatT = sb.tile([128, K], f32)  # rows 32g+(0..3) = (ax, ay, az, 1)
            bhat = sb.tile([128, K], f32)   # rows 32g+(0..3) = (-2bx,..., b2)
            So = sb.tile([128, 128], f32)
            Sa = sb.tile([128, 128], f32)
            with tc.tile_pool(name="tp", bufs=2, space="PSUM") as tp:
                for src, dst, S, eng in ((Ov, bhat, So, nc.scalar),
                                         (Av, ahatT, Sa, nc.vector)):
                    tpt = tp.tile([128, 128], f32, tag="tpt")
                    nc.tensor.transpose(tpt[:], src[:], ident[:])
                    eng.copy(S[:], tpt[:]) if eng is nc.scalar else \
                        nc.vector.tensor_copy(S[:], tpt[:])
                    # S element (32q+t, p) -> dst row q, col 128t+p
                    for q in range(4):
                        dst_v = dst[q:q + 1, :].rearrange(
                            "q (t p) -> q t p", p=128)
                        nc.sync.dma_start(out=dst_v[:],
                                          in_=S[32 * q:32 * (q + 1), :])
                    for r in (32, 64, 96):
                        nc.sync.dma_start(out=dst[r:r + 4, :],
                                          in_=dst[0:4, :])

            # Main loop: per k-tile, 8 matmuls of [4,128]^T @ [4,512] into
            # PSUM, min-reduced over j in two [128, 2048] halves.
            gminP = sb.tile([128, 2 * KT], f32)
            with tc.tile_pool(name="mm", bufs=2, space="PSUM") as mm:
                for t in range(KT):
                    for h in range(2):
                        ps = mm.tile([128, 2048], f32, tag="ps")
                        for g in range(4):
                            j0 = (h * 4 + g) * 512
                            r = 32 * g
                            nc.tensor.matmul(
                                ps[:, g * 512:(g + 1) * 512],
                                ahatT[r:r + 4, t * 128:(t + 1) * 128],
                                bhat[r:r + 4, j0:j0 + 512],
                                start=True, stop=True,
                                tile_position=(r, 0),
                            )
                        c = 2 * t + h
                        nc.vector.tensor_reduce(
                            gminP[:, c:c + 1], ps[:], axis=Ax.X, op=Alu.min)

                # Combine: min over the two halves, add a2 per point, sum.
                gmin2 = sb.tile([128, KT], f32)
                tot = sb.tile([128, KT], f32)
                ksum = sb.tile([128, 1], f32)
                res = sb.tile([1, 1], f32)
                gminP_v = gminP[:].rearrange("p (t h) -> p t h", h=2)
                nc.vector.tensor_reduce(gmin2[:], gminP_v, axis=Ax.X,
                                        op=Alu.min)
                nc.vector.tensor_tensor(tot[:], gmin2[:], a2arr[:],
                                        op=Alu.add)
                nc.vector.tensor_reduce(ksum[:], tot[:], axis=Ax.X,
                                        op=Alu.add)
                ps = mm.tile([128, 2048], f32, tag="ps")
                nc.tensor.matmul(ps[:1, :1], ksum[:], Av[:, 3 * KT:3 * KT + 1],
                                 start=True, stop=True)
                nc.scalar.copy(res[:], ps[:1, :1])
                nc.sync.dma_start(out=out[:], in_=res[:])

    nc.compile()
    return nc


def _get_nc():
    if "nc" not in _NC_CACHE:
        _NC_CACHE["nc"] = _build_nc()
    return _NC_CACHE["nc"]


def kernel(adv_pc, ori_pc, weights):
    from concourse.bass_utils import run_bass_kernel_spmd

    adv_pc = np.asarray(adv_pc, dtype=np.float32)
    ori_pc = np.asarray(ori_pc, dtype=np.float32)
    weights = np.asarray(weights, dtype=np.float32)

    nc = _get_nc()
    in_maps = [
        {"adv": np.ascontiguousarray(adv_pc[b]),
         "ori": np.ascontiguousarray(ori_pc[b])}
        for b in range(B)
    ]
    res = run_bass_kernel_spmd(nc, in_maps, core_ids=list(range(NCORES)))
    sums = np.array([res.results[b]["out"][0, 0] for b in range(B)],
                    dtype=np.float32)
    loss1 = sums / np.float32(K)
    return np.array(np.mean(loss1 * weights), dtype=np.float32)


if __name__ == "__main__":
    rng = np.random.default_rng(0)
    a = rng.standard_normal((B, K, 3), dtype=np.float32)
    o = rng.standard_normal((B, K, 3), dtype=np.float32)
    w = np.ones((B,), dtype=np.float32)
    print(kernel(a, o, w))


# revision 18
# speedup vs baseline: 1.0982x; 1.0982x over previous
"""Chamfer distance (adv->ori direction) Trainium2 Bass kernel.

Problem: adv_pc [8, 4096, 3], ori_pc [8, 4096, 3], weights [8] ->
scalar f32 loss = mean_b( w_b * mean_k( min_j ||adv_bk - ori_bj||^2 ) ).

Sharding: data parallel over the batch dim — core b handles batch b.

Per-core algorithm (K = 4096 points, d = 3):
  m[k, j]   = b2_j - 2 a_k . b_j          (augmented matmul, contract dim 4)
  out_core  = sum_k ( a2_k + min_j m[k,j] )       (= 4096 * loss1_b)
The a2_k term is added per-point BEFORE the sum over k (the min is ~ -3.0
and a2 ~ +3.0; their sum is ~0.002, so summing them separately would lose
precision to cancellation).

Matmul form: lhsT = ahatT [4, 128]   rows (ax, ay, az, 1)   per k-tile
             rhs  = bhat  [4, 4096]  rows (-2bx, -2by, -2bz, b2)
             psum[t] = lhsT.T @ rhs  -> [128, j]   then DVE min-reduce over j.

The [coord, point] layouts are built on-chip with PE transposes of
[128 points, 4] tiles (identity-matmul), scaled during the PSUM->SBUF
copy on the Scalar engine (per-partition scale (-2,-2,-2,1) for ori).
"""

import numpy as np

B = 8
K = 4096
KT = K // 128  # 32 k-tiles of 128 adv points
NCORES = 8

_NC_CACHE = {}


def _build_nc():
    import concourse.bacc as bacc
    import concourse.mybir as mybir
    import concourse.tile as tile
    from concourse import masks

    f32 = mybir.dt.float32
    Alu = mybir.AluOpType
    Act = mybir.ActivationFunctionType
    Ax = mybir.AxisListType

    nc = bacc.Bacc("TRN2", target_bir_lowering=False, debug=False,
                   num_devices=NCORES)

    adv = nc.dram_tensor("adv", [K, 3], f32, kind="ExternalInput").ap()
    ori = nc.dram_tensor("ori", [K, 3], f32, kind="ExternalInput").ap()
    out = nc.dram_tensor("out", [1, 1], f32, kind="ExternalOutput").ap()

    with tile.TileContext(nc) as tc:
        with tc.tile_pool(name="consts", bufs=1) as consts, \
             tc.tile_pool(name="sb", bufs=1) as sb:
            ident = consts.tile([128, 128], f32)
            masks.make_identity(nc, ident[:])

            # Point-major landing tiles (DMA-friendly: 12B contiguous per
            # point), then a strided DVE copy into coord-major staging
            # tiles [128, 128]: col 32q+t = coord q of k-tile t (q=3 col
            # block: 1.0 for adv, b2 for ori). Point (128t + p) lives at
            # row p, cols {t, 32+t, 64+t, 96+t}.
            Pa = sb.tile([128, 3 * KT], f32)
            Po = sb.tile([128, 3 * KT], f32)
            Av = sb.tile([128, 4 * KT], f32)
            Ov = sb.tile([128, 4 * KT], f32)
            nc.gpsimd.memset(Av[:, 3 * KT:], 1.0)

            adv_v = adv.rearrange("(t p) d -> p t d", p=128)
            ori_v = ori.rearrange("(t p) d -> p t d", p=128)
            nc.sync.dma_start(out=Pa[:].rearrange("p (t d) -> p t d", d=3),
                              in_=adv_v[:])
            nc.gpsimd.dma_start(out=Po[:].rearrange("p (t d) -> p t d", d=3),
                                in_=ori_v[:])
            Av_v = Av[:, 0:3 * KT].rearrange("p (q t) -> p t q", q=3)
            Ov_v = Ov[:, 0:3 * KT].rearrange("p (q t) -> p t q", q=3)
            nc.vector.tensor_copy(Av_v,
                                  Pa[:].rearrange("p (t d) -> p t d", d=3))
            nc.vector.tensor_copy(Ov_v,
                                  Po[:].rearrange("p (t d) -> p t d", d=3))

            # a2 per adv point -> a2arr [128, 32]; b2 per ori point -> col
            # 96+t of Ov (reduce over the stride-32 coord axis).
            Asq = sb.tile([128, 3 * KT], f32)
            Osq = sb.tile([128, 3 * KT], f32)
            a2arr = sb.tile([128, KT], f32)
            nc.vector.tensor_tensor(Asq[:], Av[:, 0:3 * KT], Av[:, 0:3 * KT],
                                    op=Alu.mult)
            nc.vector.tensor_tensor(Osq[:], Ov[:, 0:3 * KT], Ov[:, 0:3 * KT],
                                    op=Alu.mult)
            Asq_v = Asq[:].rearrange("p (q t) -> p t q", q=3)
            Osq_v = Osq[:].rearrange("p (q t) -> p t q", q=3)
            nc.vector.tensor_reduce(a2arr[:], Asq_v, axis=Ax.X, op=Alu.add)
            nc.vector.tensor_reduce(Ov[:, 3 * KT:], Osq_v, axis=Ax.X,
                                    op=Alu.add)
            # scale ori coords in place by -2 (b2 columns stay unscaled)
            nc.vector.tensor_scalar_mul(Ov[:, 0:3 * KT], Ov[:, 0:3 * KT],
                                        -2.0)

            # One PE transpose per tensor -> PSUM [128, 128] whose row 32q+t
            # holds coord q of k-tile t; engine-copy to SBUF, then DMAs
            # gather rows into the [4, K] operand layout and replicate it to
            # PE row groups 0/32/64/96 so four matmuls can run concurrently
            # via tile_position.
            ahatT = sb.tile([128, K], f32)  # rows 32g+(0..3) = (ax, ay, az, 1)
            bhat = sb.tile([128, K], f32)   # rows 32g+(0..3) = (-2bx,..., b2)
            So = sb.tile([128, 128], f32)
            Sa = sb.tile([128, 128], f32)
            dma_engs = (nc.sync, nc.gpsimd, nc.scalar)
            with tc.tile_pool(name="tp", bufs=2, space="PSUM") as tp:
                for src, dst, S, copy_eng in ((Ov, bhat, So, nc.scalar),
                                              (Av, ahatT, Sa, nc.vector)):
                    tpt = tp.tile([128, 128], f32, tag="tpt")
                    nc.tensor.transpose(tpt[:], src[:], ident[:])
                    if copy_eng is nc.scalar:
                        nc.scalar.copy(S[:], tpt[:])
                    else:
                        nc.vector.tensor_copy(S[:], tpt[:])
                    # S element (32q+t, p) -> dst row 32g+q, col 128t+p;
                    # 16 independent DMAs spread over 4 trigger engines.
                    for q in range(4):
                        for gi, r in enumerate((0, 32, 64, 96)):
                            dst_v = dst[r + q:r + q + 1, :].rearrange(
                                "q (t p) -> q t p", p=128)
                            dma_engs[(q + gi) % 3].dma_start(
                                out=dst_v[:],
                                in_=S[32 * q:32 * (q + 1), :])

            # Main loop: per k-tile, 8 matmuls of [4,128]^T @ [4,512] into
            # PSUM, min-reduced over j in two [128, 2048] halves.
            gminP = sb.tile([128, 2 * KT], f32)
            with tc.tile_pool(name="mm", bufs=2, space="PSUM") as mm:
                for t in range(KT):
                    for h in range(2):
                        ps = mm.tile([128, 2048], f32, tag="ps")
                        for g in range(4):
                            j0 = (h * 4 + g) * 512
                            r = 32 * g
                            nc.tensor.matmul(
                                ps[:, g * 512:(g + 1) * 512],
                                ahatT[r:r + 4, t * 128:(t + 1) * 128],
                                bhat[r:r + 4, j0:j0 + 512],
                                start=True, stop=True,
                                tile_position=(r, 0),
                            )
                        c = 2 * t + h
                        nc.vector.tensor_reduce(
                            gminP[:, c:c + 1], ps[:], axis=Ax.X, op=Alu.min)

                # Combine: min over the two halves, add a2 per point, sum.
                gmin2 = sb.tile([128, KT], f32)
                tot = sb.tile([128, KT], f32)
                ksum = sb.tile([128, 1], f32)
                res = sb.tile([1, 1], f32)
                gminP_v = gminP[:].rearrange("p (t h) -> p t h", h=2)
                nc.vector.tensor_reduce(gmin2[:], gminP_v, axis=Ax.X,
                                        op=Alu.min)
                nc.vector.tensor_tensor(tot[:], gmin2[:], a2arr[:],
                                        op=Alu.add)
                nc.vector.tensor_reduce(ksum[:], tot[:], axis=Ax.X,
                                        op=Alu.add)
                ps = mm.tile([128, 2048], f32, tag="ps")
                nc.tensor.matmul(ps[:1, :1], ksum[:], Av[:, 3 * KT:3 * KT + 1],
                                 start=True, stop=True)
                nc.scalar.copy(res[:], ps[:1, :1])
                nc.sync.dma_start(out=out[:], in_=res[:])

    nc.compile()
    return nc


def _get_nc():
    if "nc" not in _NC_CACHE:
        _NC_CACHE["nc"] = _build_nc()
    return _NC_CACHE["nc"]


def kernel(adv_pc, ori_pc, weights):
    from concourse.bass_utils import run_bass_kernel_spmd

    adv_pc = np.asarray(adv_pc, dtype=np.float32)
    ori_pc = np.asarray(ori_pc, dtype=np.float32)
    weights = np.asarray(weights, dtype=np.float32)

    nc = _get_nc()
    in_maps = [
        {"adv": np.ascontiguousarray(adv_pc[b]),
         "ori": np.ascontiguousarray(ori_pc[b])}
        for b in range(B)
    ]
    res = run_bass_kernel_spmd(nc, in_maps, core_ids=list(range(NCORES)))
    sums = np.array([res.results[b]["out"][0, 0] for b in range(B)],
                    dtype=np.float32)
    loss1 = sums / np.float32(K)
    return np.array(np.mean(loss1 * weights), dtype=np.float32)


if __name__ == "__main__":
    rng = np.random.default_rng(0)
    a = rng.standard_normal((B, K, 3), dtype=np.float32)
    o = rng.standard_normal((B, K, 3), dtype=np.float32)
    w = np.ones((B,), dtype=np.float32)
    print(kernel(a, o, w))
